# revision 45
# baseline (speedup 1.0000x reference)
"""Edge-parallel NNConv (CellNet) kernel for 8 Trainium2 NeuronCores.

Strategy (v4)
-------------
Nodes are LPT-assigned to cores (6250 each), then 2D bin-packed into 64
blocks per core so that each block's in-edges split by source QUARTET
(cores 0-3 vs 4-7) both fit in 256 slots (2 tiles of 128).  Tile space:
[Q0 tiles 0..127 | Q1 tiles 128..255]; block r owns Q0 tiles {2r,2r+1}
and Q1 tiles {128+2r, 128+2r+1}.

Per layer, per core:
  1. z-matmul (PE): z[e,:] = SO[:,e]^T @ C123 per 128-edge tile.
  2. relu*x (DVE STT or Act+DVE): p[e,(o,i)] = max(z,0)*x_src[e,i].
  3. segment-sum via PE: agg[m,(o,i)] += ST^T[e,m] p[e,(o,i)].
  4. i-reduction (DVE), 1/deg scale, root+bias via per-block PE matmuls
     on transposed activations (bf16), relu -> h.
  5. AllGather of the compact per-core h chunk; copy into an SBUF table
     [128p, (c q)*10]; layers 2-3 fetch x_src with SBUF-source extended
     dma_gather instructions (1024 int16 idxs each, idx = local-rank*128
     + partition, rank = (c%4)*64+q) spread over 4 SWDGE queues that run
     on distinct Q7 pairs concurrently.
  6. Final layer: masked per-graph mean pooling via PE matmuls, an
     [8,16] AllReduce, and a 1/count scale.

x_src for layer 1 is host-gathered (x is an input), SBUF-resident.
"""

import math

import numpy as np
import ml_dtypes

N_CORES = 8
LAYER_DIMS = [(16, 10), (10, 10), (10, 16)]
N_ET = 25
K_SO = 3 * N_ET
PSUM_BANK_F32 = 512
N_BLK = 64                # blocks per core
TQ = 2                    # tiles per (block, quartet)
N_QRT = 2                 # quartet groups (4 cores each)
G_TILES = 8               # tiles per gather instruction (1024 idxs)
N_QUEUES = 4


class Cfg:
    def __init__(self, n_nodes, n_edges, npc):
        self.n_nodes = n_nodes
        self.n_edges = n_edges
        self.npc = npc                        # nodes per core
        self.nblk = N_BLK
        self.npad = 128 * N_BLK               # node slots per core
        self.tpb = TQ * N_QRT                 # tiles per block (2 + 2)
        self.ntiles = N_BLK * self.tpb        # 256
        self.half = N_BLK * TQ                # tiles per quartet range: 128
        self.epad = self.ntiles * 128
        # gather instruction plan: small first rounds so the first
        # blocks' STT can start right after the AllGather lands
        sizes = [2, 2, 4] + [8] * 15          # 128 tiles per quartet
        assert sum(sizes) == self.half
        per_q = []
        for qrt in range(N_QRT):
            t = 0
            for g in sizes:
                per_q.append((qrt * self.half + t, g, qrt))
                t += g
        n = len(per_q) // N_QRT
        plan = []
        for k in range(n):
            plan.append(per_q[k])
            plan.append(per_q[k + n])
        coloff = [0] * N_QUEUES
        self.gplan = []
        for i, (t0, g, qrt) in enumerate(plan):
            q = i % N_QUEUES
            icols = g * 128 // 16
            self.gplan.append((t0, g, qrt, q, coloff[q]))
            coloff[q] += icols
        self.idx_cols = max(coloff)


_BUILD_CACHE = {}


def _dma_gather_sbuf(gp, out_ap, in_ap, idxs_ap, num_idxs, elem_size,
                     queue_num, tokens_per_rank, free_dim_per_rank):
    """SBUF-source extended dma_gather (bass asserts bypassed: 20B rows,
    non-transpose SBUF source are fine at the firmware level)."""
    import concourse.mybir as mybir

    _in_ap = gp.lower_ap(in_ap)
    _idxs_ap = gp.lower_ap(idxs_ap)
    _out_ap = gp.lower_ap(out_ap)
    return gp.add_instruction(
        mybir.InstDMAGatherAnt(
            name=gp.bass.get_next_instruction_name(),
            ins=[_in_ap, _idxs_ap,
                 gp.lower_val_access(gp.to_reg(num_idxs))],
            outs=[_out_ap],
            transpose=False,
            num_idxs=num_idxs,
            elem_size=elem_size,
            stride_bytes_256=0,
            gen_mode=0,
            single_packet=True,
            queue_num=queue_num,
            sbuf_tokens_per_rank=tokens_per_rank,
            sbuf_free_dim_per_rank=free_dim_per_rank,
            sbuf_free_dim_pad_per_rank=0,
            sbuf_byte_offset=0,
        )
    )


def build_nc(cfg: Cfg, debug=False):
    import concourse.bacc as bacc
    import concourse.bass as bass  # noqa: F401
    import concourse.mybir as mybir
    import concourse.tile as tile
    from concourse.masks import make_identity

    f32 = mybir.dt.float32
    bf16 = mybir.dt.bfloat16
    fp8 = mybir.dt.float8e4
    i16 = mybir.dt.int16
    AX = mybir.AxisListType
    OP = mybir.AluOpType

    nc = bacc.Bacc("TRN2", target_bir_lowering=False, debug=False,
                   num_devices=N_CORES, num_swdge_queues=N_QUEUES)

    nblk, npad, epad = cfg.nblk, cfg.npad, cfg.epad
    ntiles, tpb, half = cfg.ntiles, cfg.tpb, cfg.half

    # ---- kernel I/O ------------------------------------------------------
    so_d = nc.dram_tensor("so", [K_SO, epad], bf16, kind="ExternalInput")
    st_d = nc.dram_tensor("st", [128, ntiles * 128], bf16,
                          kind="ExternalInput")
    xs1_d = nc.dram_tensor("xs1", [128, ntiles * 16], bf16,
                           kind="ExternalInput")
    gidx_d = nc.dram_tensor("gidx", [128, cfg.idx_cols], i16,
                            kind="ExternalInput")
    c123_d = [nc.dram_tensor(f"c123_{l}", [K_SO, ic * oc], bf16,
                             kind="ExternalInput")
              for l, (ic, oc) in enumerate(LAYER_DIMS)]
    rb_d = [nc.dram_tensor(f"rb_{l}", [ic + 1, oc], bf16,
                           kind="ExternalInput")
            for l, (ic, oc) in enumerate(LAYER_DIMS)]
    xt1_d = nc.dram_tensor("xt1", [17, npad], bf16, kind="ExternalInput")
    ppool_d = nc.dram_tensor("ppool", [128, nblk * 8], bf16,
                             kind="ExternalInput")
    cntr_d = nc.dram_tensor("cntr", [8, 1], f32, kind="ExternalInput")
    msum_d = nc.dram_tensor("msum", [N_CORES * 8, 8], bf16,
                            kind="ExternalInput")
    out_d = nc.dram_tensor("out", [8, 16], f32, kind="ExternalOutput")
    if debug:
        dbg_h = [nc.dram_tensor(f"dbg_h{l}", [128, nblk * LAYER_DIMS[l][1]],
                                bf16, kind="ExternalOutput") for l in range(3)]
        dbg_xga = [nc.dram_tensor(f"dbg_xga{l}", [128, ntiles * 10],
                                  bf16, kind="ExternalOutput")
                   for l in range(1, 3)]

    groups = [list(range(N_CORES))]

    with tile.TileContext(nc) as tc:
        with (
            tc.tile_pool(name="res", bufs=1) as res,
            tc.tile_pool(name="xga", bufs=1) as xga_pool,
            tc.tile_pool(name="pb", bufs=2) as pb_pool,
            tc.tile_pool(name="node", bufs=1) as node_pool,
            tc.tile_pool(name="small", bufs=2) as small_pool,
            tc.tile_pool(name="zp", bufs=3, space="PSUM") as zp_pool,
            tc.tile_pool(name="ag", bufs=2, space="PSUM") as ag_pool,
            tc.tile_pool(name="rt", bufs=1, space="PSUM") as rt_pool,
            tc.tile_pool(name="tp", bufs=1, space="PSUM") as tp_pool,
            tc.tile_pool(name="dram", bufs=1, space="DRAM") as dram,
        ):
            # ---- residents ---------------------------------------------
            # chunk-interleave the big loads so block-0 compute starts
            # after ~1 MB (HWDGE drains FIFO in issue order)
            st_sb = res.tile([128, ntiles * 128], bf16)
            so_sb = res.tile([K_SO, epad], bf16)
            xs1_sb = res.tile([128, ntiles * 16], bf16)
            c123_sb = []
            for l in range(3):
                t = res.tile([K_SO, LAYER_DIMS[l][0] * LAYER_DIMS[l][1]],
                             bf16, tag=f"c123_{l}", name=f"c123s{l}")
                nc.sync.dma_start(out=t[:], in_=c123_d[l].ap())
                c123_sb.append(t)
            rb_sb = []
            for l in range(3):
                ic, oc = LAYER_DIMS[l]
                t = res.tile([ic + 1, oc], bf16, tag=f"rb_{l}",
                             name=f"rbs{l}")
                nc.sync.dma_start(out=t[:], in_=rb_d[l].ap())
                rb_sb.append(t)
            CH = 40                          # tiles per resident chunk
            for t0 in range(0, ntiles, CH):
                t1 = min(ntiles, t0 + CH)
                nc.sync.dma_start(out=so_sb[:, t0 * 128:t1 * 128],
                                  in_=so_d.ap()[:, t0 * 128:t1 * 128])
                nc.sync.dma_start(out=xs1_sb[:, t0 * 16:t1 * 16],
                                  in_=xs1_d.ap()[:, t0 * 16:t1 * 16])
                nc.sync.dma_start(out=st_sb[:, t0 * 128:t1 * 128],
                                  in_=st_d.ap()[:, t0 * 128:t1 * 128])
            gidx_sb = res.tile([128, cfg.idx_cols], i16)
            nc.sync.dma_start(out=gidx_sb[:], in_=gidx_d.ap())
            xt_sb = res.tile([17, npad], bf16)      # x^T / h^T + ones row
            nc.sync.dma_start(out=xt_sb[:], in_=xt1_d.ap())
            ppool_sb = res.tile([128, nblk * 8], bf16)
            nc.sync.dma_start(out=ppool_sb[:], in_=ppool_d.ap())
            cntr_sb = res.tile([8, 1], f32)
            nc.sync.dma_start(out=cntr_sb[:], in_=cntr_d.ap())
            msum_sb = res.tile([N_CORES * 8, 8], bf16)
            nc.sync.dma_start(out=msum_sb[:], in_=msum_d.ap())
            ident = res.tile([128, 128], bf16)
            make_identity(nc, ident[:])
            hf_sb = res.tile([128, N_CORES * nblk * 10], bf16)

            # PE warm-up: dependency-free matmuls issued while the resident
            # DMAs stream in, so the HAM clock-gate opens before layer 1
            warm = rt_pool.tile([128, 128], f32, tag="rt", name="warm")
            for _ in range(200):
                nc.tensor.matmul(out=warm[:], lhsT=ident[:], rhs=ident[:],
                                 start=True, stop=True)

            # ---- DRAM scratch ------------------------------------------
            hc = [dram.tile([128, nblk * 10], bf16, tag=f"hc{l}",
                            name=f"hc{l}") for l in range(2)]
            hf = [dram.tile([N_CORES, 128, nblk * 10], bf16, tag=f"hf{l}",
                            name=f"hf{l}", addr_space="Shared")
                  for l in range(2)]
            pool_in = dram.tile([8, 16], bf16)
            pool_out = dram.tile([N_CORES, 8, 16], bf16,
                                 addr_space="Shared")

            # so/st/xs1 tiles are block-contiguous (DMA streaming order);
            # xga tiles are quartet-half ordered (gather instruction order)
            def tiles_of_block(r):
                return [4 * r, 4 * r + 1, 4 * r + 2, 4 * r + 3]

            def xga_tiles_of_block(r):
                return [2 * r, 2 * r + 1, half + 2 * r, half + 2 * r + 1]

            for l in range(3):
                ic, oc = LAYER_DIMS[l]
                d = ic * oc
                gmax = PSUM_BANK_F32 // d

                # ---- x_src for this layer ------------------------------
                if l == 0:
                    xg = xs1_sb[:].rearrange("p (t i) -> p t i", i=16)
                else:
                    xga = xga_pool.tile([128, ntiles * 10], bf16,
                                        tag="xga")
                    for c in range(N_CORES):
                        nc.sync.dma_start(
                            out=hf_sb[:, c * nblk * 10:(c + 1) * nblk * 10],
                            in_=hf[l - 1][c])
                    for (t0, g, qrt, q, co) in cfg.gplan:
                        icols = g * 128 // 16
                        _dma_gather_sbuf(
                            nc.gpsimd,
                            out_ap=xga[:, t0 * 10:(t0 + g) * 10],
                            in_ap=hf_sb[:, qrt * 4 * nblk * 10:
                                        (qrt + 1) * 4 * nblk * 10],
                            idxs_ap=gidx_sb[0:16, co:co + icols],
                            num_idxs=g * 128,
                            elem_size=10,
                            queue_num=q,
                            tokens_per_rank=128,
                            free_dim_per_rank=20)
                    if debug:
                        nc.sync.dma_start(out=dbg_xga[l - 1].ap(),
                                          in_=xga[:])
                    xg = xga[:].rearrange("p (t i) -> p t i", i=10)

                asum = node_pool.tile([128, nblk * oc], f32, tag="asum")
                prev = None  # (pbuf, r) pending scatter
                B = PSUM_BANK_F32 // d  # blocks per scatter-PSUM bank
                sc = {"agp": None, "blocks": []}

                def emit_scatter(pb_r, rp):
                    if sc["agp"] is None:
                        sc["agp"] = ag_pool.tile([128, B * d], f32,
                                                 tag="ag", name="agp")
                        sc["blocks"] = []
                    b = len(sc["blocks"])
                    agp = sc["agp"]
                    tls = tiles_of_block(rp)
                    for j, tl in enumerate(tls):
                        nc.tensor.matmul(
                            out=agp[:, b * d:(b + 1) * d],
                            lhsT=st_sb[:, tl * 128:(tl + 1) * 128],
                            rhs=pb_r[:, j * d:(j + 1) * d],
                            start=(j == 0), stop=(j == len(tls) - 1))
                    sc["blocks"].append(rp)
                    if len(sc["blocks"]) == B or rp == nblk - 1:
                        nb = len(sc["blocks"])
                        r0 = sc["blocks"][0]
                        nc.vector.reduce_sum(
                            out=asum[:, r0 * oc:(r0 + nb) * oc],
                            in_=agp[:, 0:nb * d].rearrange(
                                "p (q i) -> p q i", i=ic),
                            axis=AX.X)
                        sc["agp"] = None

                for r in range(nblk):
                    lane = (0, 2)[r % 2]
                    tls = tiles_of_block(r)
                    xtls = tls if l == 0 else xga_tiles_of_block(r)
                    pbuf = pb_pool.tile([128, tpb * d], bf16, tag="pb")
                    if lane != 0:
                        rbuf = pb_pool.tile([128, tpb * d], bf16, tag="rb")
                    t0 = 0
                    while t0 < tpb:
                        g = min(gmax, tpb - t0)
                        zp = zp_pool.tile([128, PSUM_BANK_F32], f32,
                                          tag="zp")
                        for j in range(g):
                            tl = tls[t0 + j]
                            nc.tensor.matmul(
                                out=zp[:, j * d:(j + 1) * d],
                                lhsT=so_sb[:, tl * 128:(tl + 1) * 128],
                                rhs=c123_sb[l][:],
                                start=True, stop=True)
                        if lane == 0:
                            for j in range(g):
                                t = t0 + j
                                nc.vector.scalar_tensor_tensor(
                                    out=pbuf[:, t * d:(t + 1) * d]
                                    .rearrange("p (o i) -> p o i",
                                               o=oc, i=ic),
                                    in0=zp[:, j * d:(j + 1) * d].rearrange(
                                        "p (o i) -> p o i", o=oc, i=ic),
                                    scalar=0.0,
                                    in1=xg[:, xtls[t]:xtls[t] + 1,
                                           :].to_broadcast([128, oc, ic]),
                                    op0=OP.max, op1=OP.mult)
                        else:
                            nc.scalar.activation(
                                out=rbuf[:, t0 * d:(t0 + g) * d],
                                in_=zp[:, 0:g * d],
                                func=mybir.ActivationFunctionType.Relu)
                            eng = nc.gpsimd if lane == 1 else nc.vector
                            for j in range(g):
                                t = t0 + j
                                eng.tensor_tensor(
                                    out=pbuf[:, t * d:(t + 1) * d]
                                    .rearrange("p (o i) -> p o i",
                                               o=oc, i=ic),
                                    in0=rbuf[:, t * d:(t + 1) * d]
                                    .rearrange("p (o i) -> p o i",
                                               o=oc, i=ic),
                                    in1=xg[:, xtls[t]:xtls[t] + 1,
                                           :].to_broadcast([128, oc, ic]),
                                    op=OP.mult)
                        t0 += g

                    # emit the PREVIOUS block's scatter so PE never
                    # head-blocks on this block's relu*x
                    if prev is not None:
                        emit_scatter(*prev)
                    prev = (pbuf, r)

                emit_scatter(*prev)
                prev = None

                # ---- node phase (batched over blocks) ------------------
                # 16-col stride per block so no matmul output crosses a
                # PSUM bank boundary (64*16 = exactly 2 banks)
                rt = rt_pool.tile([128, nblk * 16], f32, tag="rt")
                kr = 17 if l == 0 else 11
                for r in range(nblk):
                    nc.tensor.matmul(
                        out=rt[:, r * 16:r * 16 + oc],
                        lhsT=xt_sb[0:kr, r * 128:(r + 1) * 128],
                        rhs=rb_sb[l][:],
                        start=True, stop=True)
                hsum = node_pool.tile([128, nblk * oc], f32, tag="hsum")
                nc.vector.scalar_tensor_tensor(
                    out=hsum[:].rearrange("p (q o) -> p q o", o=oc),
                    in0=rt[:].rearrange("p (q s) -> p q s", s=16)
                    [:, :, 0:oc],
                    scalar=0.0,
                    in1=asum[:].rearrange("p (q o) -> p q o", o=oc),
                    op0=OP.add, op1=OP.add)
                hrel = node_pool.tile([128, nblk * oc], bf16, tag="hrel")
                nc.vector.tensor_scalar_max(out=hrel[:], in0=hsum[:],
                                            scalar1=0.0)
                if debug:
                    nc.sync.dma_start(out=dbg_h[l].ap(), in_=hrel[:])

                if l < 2:
                    nc.sync.dma_start(out=hc[l][:], in_=hrel[:])
                    nc.gpsimd.collective_compute(
                        "AllGather", OP.bypass, replica_groups=groups,
                        ins=[hc[l][:]], outs=[hf[l][:]])
                    for r in range(nblk):
                        tp = tp_pool.tile([16, 128], bf16, tag="tp")
                        nc.tensor.transpose(
                            out=tp[0:oc, :],
                            in_=hrel[:, r * oc:(r + 1) * oc],
                            identity=ident[:])
                        nc.scalar.copy(
                            out=xt_sb[0:oc, r * 128:(r + 1) * 128],
                            in_=tp[0:oc, :])
                else:
                    plp = tp_pool.tile([8, 16], f32, tag="tp", name="plp")
                    for r in range(nblk):
                        nc.tensor.matmul(
                            out=plp[:],
                            lhsT=ppool_sb[:, r * 8:(r + 1) * 8],
                            rhs=hrel[:, r * oc:(r + 1) * oc],
                            start=(r == 0), stop=(r == nblk - 1))
                    pool_sb = small_pool.tile([8, 16], bf16, tag="pool_sb")
                    nc.scalar.copy(out=pool_sb[:], in_=plp[:])
                    nc.sync.dma_start(out=pool_in[:], in_=pool_sb[:])
                    nc.gpsimd.collective_compute(
                        "AllGather", OP.bypass, replica_groups=groups,
                        ins=[pool_in[:]], outs=[pool_out[:]])
                    pool2 = small_pool.tile([N_CORES * 8, 16], bf16,
                                            tag="pool2")
                    nc.sync.dma_start(
                        out=pool2[:],
                        in_=pool_out[:].rearrange("c a b -> (c a) b"))
                    pool3 = tp_pool.tile([8, 16], f32, tag="tp",
                                         name="pool3")
                    nc.tensor.matmul(out=pool3[:], lhsT=msum_sb[:],
                                     rhs=pool2[:], start=True, stop=True)
                    outp = small_pool.tile([8, 16], f32, tag="outp")
                    nc.vector.tensor_scalar_mul(out=outp[:], in0=pool3[:],
                                                scalar1=cntr_sb[:, 0:1])
                    nc.sync.dma_start(out=out_d.ap(), in_=outp[:])

    nc.compile()
    return nc


# --------------------------------------------------------------------------
# host-side preparation
# --------------------------------------------------------------------------

def _assign_nodes(deg, n, npc):
    """LPT: nodes -> cores balancing in-edge load, <= npc nodes/core."""
    import heapq

    order = np.argsort(-deg, kind="stable")
    heap = [(0, c) for c in range(N_CORES)]
    heapq.heapify(heap)
    counts = np.zeros(N_CORES, np.int64)
    core_of = np.zeros(n, np.int64)
    spill = []
    for node in order:
        load, c = heapq.heappop(heap)
        core_of[node] = c
        counts[c] += 1
        load += int(deg[node])
        if counts[c] < npc:
            heapq.heappush(heap, (load, c))
        else:
            spill.append((load, c))
    return core_of


def _pack_blocks(nodes, wa, wb, nblk, cap, max_nodes):
    """Greedy 2D packing: nodes (sorted by wa+wb desc) into nblk blocks
    with per-block quartet loads <= cap and <= max_nodes nodes.
    Returns block assignment or None on failure."""
    la = np.zeros(nblk, np.int64)
    lb = np.zeros(nblk, np.int64)
    cnt = np.zeros(nblk, np.int64)
    blk_of = np.zeros(len(nodes), np.int64)
    order = np.argsort(-(wa + wb), kind="stable")
    for i in order:
        a, b = wa[i], wb[i]
        feas = (la + a <= cap) & (lb + b <= cap) & (cnt < max_nodes)
        if not feas.any():
            return None
        score = np.maximum(la + a, lb + b)
        score[~feas] = 1 << 60
        j = int(np.argmin(score))
        blk_of[i] = j
        la[j] += a
        lb[j] += b
        cnt[j] += 1
    return blk_of


def host_prep(cfg: Cfg, inputs: dict, aux=None):
    np_f32 = np.float32
    bf = ml_dtypes.bfloat16
    x = np.asarray(inputs["x"], np_f32)
    ef = np.asarray(inputs["edge_feat"], np_f32)
    et = np.asarray(inputs["edge_type"]).astype(np.int64)
    src = np.asarray(inputs["edge_src"]).astype(np.int64)
    dst = np.asarray(inputs["edge_dst"]).astype(np.int64)
    cell = np.asarray(inputs["cell_type"]).astype(np.int64)
    bids = np.asarray(inputs["batch_ids"]).astype(np.int64)

    n = cfg.n_nodes
    npad, nblk, tpb = cfg.npad, cfg.nblk, cfg.tpb
    ntiles, half, epad = cfg.ntiles, cfg.half, cfg.epad

    deg = np.bincount(dst, minlength=n).astype(np.int64)
    ncore, npart, nr = (inputs["_node_core"], inputs["_node_p"],
                        inputs["_node_r"])

    # folded weights per layer (o-major columns) + root/bias stack
    c123, rbs = [], []
    for l, (ic, oc) in enumerate(LAYER_DIMS):
        i = l + 1
        emb = np.asarray(inputs[f"emb{i}"], np_f32)
        wh = np.asarray(inputs[f"wh{i}"], np_f32)
        bh = np.asarray(inputs[f"bh{i}"], np_f32)
        wg = np.asarray(inputs[f"wg{i}"], np_f32)
        bg = np.asarray(inputs[f"bg{i}"], np_f32)
        c1 = emb * wh[0][None, :] + wg[0][None, :]
        c2 = emb * wh[1][None, :] + wg[1][None, :]
        c3 = emb * bh[None, :] + bg[None, :]
        m = np.concatenate([c1, c2, c3], axis=0)
        j = np.arange(ic * oc)
        o_, i_ = j // ic, j % ic
        c123.append(np.ascontiguousarray(m[:, i_ * oc + o_]).astype(bf))
        root = np.asarray(inputs[f"root{i}"], np_f32)
        bias = np.asarray(inputs[f"bias{i}"], np_f32)
        if l == 0:
            # xt row layout: feat 0-9, ones (row 10), feat 10-15 — so the
            # ones row is already in place for the layer 2-3 root matmuls
            rbs.append(np.vstack([root[0:10], bias[None, :],
                                  root[10:16]]).astype(bf))
        else:
            rbs.append(np.vstack([root, bias[None, :]]).astype(bf))

    e_core = ncore[dst]
    e_r = nr[dst]
    e_p = npart[dst]
    e_qrt = (ncore[src] // 4).astype(np.int64)

    # gather idx of a source node: local rank*(128) + partition
    g_idx = (((ncore[src] % 4) * nblk + nr[src]) * 128
             + npart[src]).astype(np.int64)

    msum = np.zeros((N_CORES * 8, 8), np.float32)
    for c in range(N_CORES):
        msum[c * 8 + np.arange(8), np.arange(8)] = 1.0
    msum = msum.astype(bf)

    gate = (cell == 1)
    cnt = np.bincount(bids[gate], minlength=8).astype(np_f32)
    cntr = (1.0 / np.maximum(cnt, 1.0)).reshape(8, 1).astype(np_f32)

    icols = G_TILES * 128 // 16

    in_maps = []
    for c in range(N_CORES):
        esel = np.where(e_core == c)[0]
        blk = e_r[esel]
        qrt = e_qrt[esel]
        # slot = tile*128 + pos within tile; tiles: Q0 block r -> 2r,2r+1
        # (positions 0..255), Q1 -> half+2r, half+2r+1
        order = np.lexsort((qrt, blk))
        esel = esel[order]
        blk = blk[order]
        qrt = qrt[order]
        # position within (block, quartet)
        key = blk * 2 + qrt
        bc = np.bincount(key, minlength=nblk * 2)
        assert bc.max() <= TQ * 128, (bc.max(), TQ * 128)
        within = np.arange(len(esel)) - np.repeat(
            np.concatenate([[0], np.cumsum(bc)[:-1]]), bc)
        tile0 = 4 * blk + 2 * qrt
        slots = (tile0 + within // 128) * 128 + within % 128

        et_c = et[esel]
        ef_c = ef[esel]

        so = np.zeros((K_SO, epad), np_f32)
        so[et_c, slots] = ef_c[:, 0]
        so[N_ET + et_c, slots] = ef_c[:, 1]
        so[2 * N_ET + et_c, slots] = 1.0

        st = np.zeros((epad, 128), np_f32)
        st[slots, e_p[esel]] = 1.0 / np.maximum(deg[dst[esel]], 1.0)
        st = st.reshape(ntiles, 128, 128).transpose(1, 0, 2)
        st = np.ascontiguousarray(st).reshape(128, ntiles * 128)

        xs = np.zeros((epad, 16), np_f32)
        xs[slots, :] = x[src[esel]]
        xs = xs.reshape(ntiles, 128, 16).transpose(1, 0, 2)
        xs = np.ascontiguousarray(xs).reshape(128, ntiles * 16)

        # gather idxs per slot (pad slots -> 0); permute from the
        # block-contiguous so/st order into xga quartet-half order
        gs = np.zeros(epad, np.int64)
        gs[slots] = g_idx[esel]
        S = np.empty(ntiles, np.int64)   # xga tile -> so tile
        rr = np.arange(nblk)
        S[2 * rr] = 4 * rr
        S[2 * rr + 1] = 4 * rr + 1
        S[half + 2 * rr] = 4 * rr + 2
        S[half + 2 * rr + 1] = 4 * rr + 3
        gs = gs.reshape(ntiles, 128)[S].ravel()
        gidx = np.zeros((128, cfg.idx_cols), np.int16)
        for (t0, g, qq, qnum, co) in cfg.gplan:
            ic_n = g * 128 // 16
            vals = gs[t0 * 128:(t0 + g) * 128]  # position i order
            w = np.zeros((16, ic_n), np.int16)
            ii = np.arange(g * 128)
            w[ii % 16, ii // 16] = vals.astype(np.int16)
            band = qnum * 32
            colr = slice(co, co + ic_n)
            gidx[band:band + 16, colr] = w
            gidx[band + 16:band + 32, colr] = w

        own = np.where(ncore == c)[0]
        ell = npart[own] * nblk + nr[own]

        xt1 = np.zeros((17, npad), np_f32)  # col r*128+p = node (p,r)
        cols = nr[own] * 128 + npart[own]
        xt1[0:10, cols] = x[own].T[0:10]
        xt1[10, :] = 1.0
        xt1[11:17, cols] = x[own].T[10:16]

        pp = np.zeros((npad, 8), np_f32)
        g = gate[own]
        pp[ell[g], bids[own][g]] = 1.0
        pp = pp.reshape(128, nblk * 8)

        if aux is not None:
            aux.append({"slots": slots, "esel": esel})
        in_maps.append({
            "so": so.astype(bf),
            "st": st.astype(bf),
            "xs1": xs.astype(bf),
            "gidx": gidx,
            "c123_0": c123[0], "c123_1": c123[1], "c123_2": c123[2],
            "rb_0": rbs[0], "rb_1": rbs[1], "rb_2": rbs[2],
            "xt1": xt1.astype(bf),
            "msum": msum,
            "ppool": pp.astype(bf),
            "cntr": cntr,
        })
    return in_maps


def prepare(inputs: dict, n_nodes, n_edges, npc):
    """Node->core->block assignment honoring per-quartet tile caps."""
    src = np.asarray(inputs["edge_src"]).astype(np.int64)
    dst = np.asarray(inputs["edge_dst"]).astype(np.int64)
    deg = np.bincount(dst, minlength=n_nodes).astype(np.int64)
    core = _assign_nodes(deg, n_nodes, npc)

    # per-node in-edge counts split by src quartet
    e_qrt = core[src] // 4
    wa_all = np.bincount(dst[e_qrt == 0], minlength=n_nodes)
    wb_all = np.bincount(dst[e_qrt == 1], minlength=n_nodes)

    p = np.zeros(n_nodes, np.int64)
    r = np.zeros(n_nodes, np.int64)
    for c in range(N_CORES):
        own = np.where(core == c)[0]
        blk = _pack_blocks(own, wa_all[own], wb_all[own], N_BLK,
                           TQ * 128, 128)
        assert blk is not None, f"2D block packing failed on core {c}"
        r[own] = blk
        # partition slot within block
        for b in range(N_BLK):
            sel = own[blk == b]
            p[sel] = np.arange(len(sel))

    cfg = Cfg(n_nodes, n_edges, npc)
    inputs = dict(inputs)
    inputs["_node_core"] = core
    inputs["_node_p"] = p
    inputs["_node_r"] = r
    return cfg, inputs


def kernel(**inputs) -> np.ndarray:
    from concourse.bass_utils import run_bass_kernel_spmd

    cfg, inputs2 = prepare(inputs, 50000, 250000, 6250)
    key = ("v4", cfg.ntiles)
    if key not in _BUILD_CACHE:
        _BUILD_CACHE[key] = build_nc(cfg)
    nc = _BUILD_CACHE[key]
    in_maps = host_prep(cfg, inputs2)
    res = run_bass_kernel_spmd(nc, in_maps, list(range(N_CORES)))
    return np.asarray(res.results[0]["out"], np.float32)


# revision 46
# speedup vs baseline: 1.1830x; 1.1830x over previous
"""Edge-parallel NNConv (CellNet) kernel for 8 Trainium2 NeuronCores.

Strategy (v4)
-------------
Nodes are LPT-assigned to cores (6250 each), then 2D bin-packed into 64
blocks per core so that each block's in-edges split by source QUARTET
(cores 0-3 vs 4-7) both fit in 256 slots (2 tiles of 128).  Tile space:
[Q0 tiles 0..127 | Q1 tiles 128..255]; block r owns Q0 tiles {2r,2r+1}
and Q1 tiles {128+2r, 128+2r+1}.

Per layer, per core:
  1. z-matmul (PE): z[e,:] = SO[:,e]^T @ C123 per 128-edge tile.
  2. relu*x (DVE STT or Act+DVE): p[e,(o,i)] = max(z,0)*x_src[e,i].
  3. segment-sum via PE: agg[m,(o,i)] += ST^T[e,m] p[e,(o,i)].
  4. i-reduction (DVE), 1/deg scale, root+bias via per-block PE matmuls
     on transposed activations (bf16), relu -> h.
  5. AllGather of the compact per-core h chunk; copy into an SBUF table
     [128p, (c q)*10]; layers 2-3 fetch x_src with SBUF-source extended
     dma_gather instructions (1024 int16 idxs each, idx = local-rank*128
     + partition, rank = (c%4)*64+q) spread over 4 SWDGE queues that run
     on distinct Q7 pairs concurrently.
  6. Final layer: masked per-graph mean pooling via PE matmuls, an
     [8,16] AllReduce, and a 1/count scale.

x_src for layer 1 is host-gathered (x is an input), SBUF-resident.
"""

import math

import numpy as np
import ml_dtypes

N_CORES = 8
LAYER_DIMS = [(16, 10), (10, 10), (10, 16)]
N_ET = 25
K_SO = 3 * N_ET
PSUM_BANK_F32 = 512
N_BLK = 64                # blocks per core
TQ = 2                    # tiles per (block, quartet)
N_QRT = 2                 # quartet groups (4 cores each)
G_TILES = 8               # tiles per gather instruction (1024 idxs)
N_QUEUES = 4


class Cfg:
    def __init__(self, n_nodes, n_edges, npc):
        self.n_nodes = n_nodes
        self.n_edges = n_edges
        self.npc = npc                        # nodes per core
        self.nblk = N_BLK
        self.npad = 128 * N_BLK               # node slots per core
        self.tpb = TQ * N_QRT                 # tiles per block (2 + 2)
        self.ntiles = N_BLK * self.tpb        # 256
        self.half = N_BLK * TQ                # tiles per quartet range: 128
        self.epad = self.ntiles * 128
        # gather instruction plan: small first rounds so the first
        # blocks' STT can start right after the AllGather lands
        sizes = [2, 2, 4] + [8] * 15          # 128 tiles per quartet
        assert sum(sizes) == self.half
        per_q = []
        for qrt in range(N_QRT):
            t = 0
            for g in sizes:
                per_q.append((qrt * self.half + t, g, qrt))
                t += g
        n = len(per_q) // N_QRT
        plan = []
        for k in range(n):
            plan.append(per_q[k])
            plan.append(per_q[k + n])
        coloff = [0] * N_QUEUES
        self.gplan = []
        for i, (t0, g, qrt) in enumerate(plan):
            q = i % N_QUEUES
            icols = g * 128 // 16
            self.gplan.append((t0, g, qrt, q, coloff[q]))
            coloff[q] += icols
        self.idx_cols = max(coloff)


_BUILD_CACHE = {}


def _dma_gather_sbuf(gp, out_ap, in_ap, idxs_ap, num_idxs, elem_size,
                     queue_num, tokens_per_rank, free_dim_per_rank):
    """SBUF-source extended dma_gather (bass asserts bypassed: 20B rows,
    non-transpose SBUF source are fine at the firmware level)."""
    import concourse.mybir as mybir

    _in_ap = gp.lower_ap(in_ap)
    _idxs_ap = gp.lower_ap(idxs_ap)
    _out_ap = gp.lower_ap(out_ap)
    return gp.add_instruction(
        mybir.InstDMAGatherAnt(
            name=gp.bass.get_next_instruction_name(),
            ins=[_in_ap, _idxs_ap,
                 gp.lower_val_access(gp.to_reg(num_idxs))],
            outs=[_out_ap],
            transpose=False,
            num_idxs=num_idxs,
            elem_size=elem_size,
            stride_bytes_256=0,
            gen_mode=0,
            single_packet=True,
            queue_num=queue_num,
            sbuf_tokens_per_rank=tokens_per_rank,
            sbuf_free_dim_per_rank=free_dim_per_rank,
            sbuf_free_dim_pad_per_rank=0,
            sbuf_byte_offset=0,
        )
    )


def build_nc(cfg: Cfg, debug=False):
    import concourse.bacc as bacc
    import concourse.bass as bass  # noqa: F401
    import concourse.mybir as mybir
    import concourse.tile as tile
    from concourse.masks import make_identity

    f32 = mybir.dt.float32
    bf16 = mybir.dt.bfloat16
    fp8 = mybir.dt.float8e4
    i16 = mybir.dt.int16
    AX = mybir.AxisListType
    OP = mybir.AluOpType

    nc = bacc.Bacc("TRN2", target_bir_lowering=False, debug=False,
                   num_devices=N_CORES, num_swdge_queues=N_QUEUES)

    nblk, npad, epad = cfg.nblk, cfg.npad, cfg.epad
    ntiles, tpb, half = cfg.ntiles, cfg.tpb, cfg.half

    # ---- kernel I/O ------------------------------------------------------
    so_d = nc.dram_tensor("so", [K_SO, epad], bf16, kind="ExternalInput")
    st_d = nc.dram_tensor("st", [128, ntiles * 128], bf16,
                          kind="ExternalInput")
    xs1_d = nc.dram_tensor("xs1", [128, ntiles * 16], bf16,
                           kind="ExternalInput")
    gidx_d = nc.dram_tensor("gidx", [128, cfg.idx_cols], i16,
                            kind="ExternalInput")
    c123_d = [nc.dram_tensor(f"c123_{l}", [K_SO, ic * oc], bf16,
                             kind="ExternalInput")
              for l, (ic, oc) in enumerate(LAYER_DIMS)]
    rb_d = [nc.dram_tensor(f"rb_{l}", [ic + 1, oc], bf16,
                           kind="ExternalInput")
            for l, (ic, oc) in enumerate(LAYER_DIMS)]
    xt1_d = nc.dram_tensor("xt1", [17, npad], bf16, kind="ExternalInput")
    ppool_d = nc.dram_tensor("ppool", [128, nblk * 8], bf16,
                             kind="ExternalInput")
    cntr_d = nc.dram_tensor("cntr", [8, 1], f32, kind="ExternalInput")
    msum_d = nc.dram_tensor("msum", [N_CORES * 8, 8], bf16,
                            kind="ExternalInput")
    out_d = nc.dram_tensor("out", [8, 16], f32, kind="ExternalOutput")
    if debug:
        dbg_h = [nc.dram_tensor(f"dbg_h{l}", [128, nblk * LAYER_DIMS[l][1]],
                                bf16, kind="ExternalOutput") for l in range(3)]
        dbg_xga = [nc.dram_tensor(f"dbg_xga{l}", [128, ntiles * 10],
                                  bf16, kind="ExternalOutput")
                   for l in range(1, 3)]

    groups = [list(range(N_CORES))]

    with tile.TileContext(nc) as tc:
        with (
            tc.tile_pool(name="res", bufs=1) as res,
            tc.tile_pool(name="xga", bufs=1) as xga_pool,
            tc.tile_pool(name="pb", bufs=2) as pb_pool,
            tc.tile_pool(name="node", bufs=1) as node_pool,
            tc.tile_pool(name="small", bufs=2) as small_pool,
            tc.tile_pool(name="zp", bufs=3, space="PSUM") as zp_pool,
            tc.tile_pool(name="ag", bufs=2, space="PSUM") as ag_pool,
            tc.tile_pool(name="rt", bufs=1, space="PSUM") as rt_pool,
            tc.tile_pool(name="tp", bufs=1, space="PSUM") as tp_pool,
            tc.tile_pool(name="dram", bufs=1, space="DRAM") as dram,
        ):
            # ---- residents ---------------------------------------------
            # chunk-interleave the big loads so block-0 compute starts
            # after ~1 MB (HWDGE drains FIFO in issue order)
            st_sb = res.tile([128, ntiles * 128], bf16)
            so_sb = res.tile([K_SO, epad], bf16)
            xs1_sb = res.tile([128, ntiles * 16], bf16)
            c123_sb = []
            for l in range(3):
                t = res.tile([K_SO, LAYER_DIMS[l][0] * LAYER_DIMS[l][1]],
                             bf16, tag=f"c123_{l}", name=f"c123s{l}")
                nc.sync.dma_start(out=t[:], in_=c123_d[l].ap())
                c123_sb.append(t)
            rb_sb = []
            for l in range(3):
                ic, oc = LAYER_DIMS[l]
                t = res.tile([ic + 1, oc], bf16, tag=f"rb_{l}",
                             name=f"rbs{l}")
                nc.sync.dma_start(out=t[:], in_=rb_d[l].ap())
                rb_sb.append(t)
            CH = 40                          # tiles per resident chunk
            for t0 in range(0, ntiles, CH):
                t1 = min(ntiles, t0 + CH)
                nc.sync.dma_start(out=so_sb[:, t0 * 128:t1 * 128],
                                  in_=so_d.ap()[:, t0 * 128:t1 * 128])
                nc.sync.dma_start(out=xs1_sb[:, t0 * 16:t1 * 16],
                                  in_=xs1_d.ap()[:, t0 * 16:t1 * 16])
                nc.sync.dma_start(out=st_sb[:, t0 * 128:t1 * 128],
                                  in_=st_d.ap()[:, t0 * 128:t1 * 128])
            gidx_sb = res.tile([128, cfg.idx_cols], i16)
            nc.sync.dma_start(out=gidx_sb[:], in_=gidx_d.ap())
            xt_sb = res.tile([17, npad], bf16)      # x^T / h^T + ones row
            nc.sync.dma_start(out=xt_sb[:], in_=xt1_d.ap())
            ppool_sb = res.tile([128, nblk * 8], bf16)
            nc.sync.dma_start(out=ppool_sb[:], in_=ppool_d.ap())
            cntr_sb = res.tile([8, 1], f32)
            nc.sync.dma_start(out=cntr_sb[:], in_=cntr_d.ap())
            msum_sb = res.tile([N_CORES * 8, 8], bf16)
            nc.sync.dma_start(out=msum_sb[:], in_=msum_d.ap())
            ident = res.tile([128, 128], bf16)
            make_identity(nc, ident[:])
            hf_sb = res.tile([128, N_CORES * nblk * 10], bf16)

            # PE warm-up: dependency-free matmuls issued while the resident
            # DMAs stream in, so the HAM clock-gate opens before layer 1
            warm = rt_pool.tile([128, 128], f32, tag="rt", name="warm")
            for _ in range(200):
                nc.tensor.matmul(out=warm[:], lhsT=ident[:], rhs=ident[:],
                                 start=True, stop=True)

            # ---- DRAM scratch ------------------------------------------
            hc = [dram.tile([128, nblk * 10], bf16, tag=f"hc{l}",
                            name=f"hc{l}") for l in range(2)]
            hf = [dram.tile([N_CORES, 128, nblk * 10], bf16, tag=f"hf{l}",
                            name=f"hf{l}", addr_space="Shared")
                  for l in range(2)]
            pool_in = dram.tile([8, 16], f32)
            pool_out = dram.tile([8, 16], f32, addr_space="Shared")

            # so/st/xs1 tiles are block-contiguous (DMA streaming order);
            # xga tiles are quartet-half ordered (gather instruction order)
            def tiles_of_block(r):
                return [4 * r, 4 * r + 1, 4 * r + 2, 4 * r + 3]

            def xga_tiles_of_block(r):
                return [2 * r, 2 * r + 1, half + 2 * r, half + 2 * r + 1]

            for l in range(3):
                ic, oc = LAYER_DIMS[l]
                d = ic * oc
                gmax = PSUM_BANK_F32 // d

                # ---- x_src for this layer ------------------------------
                if l == 0:
                    xg = xs1_sb[:].rearrange("p (t i) -> p t i", i=16)
                else:
                    xga = xga_pool.tile([128, ntiles * 10], bf16,
                                        tag="xga")
                    for c in range(N_CORES):
                        nc.sync.dma_start(
                            out=hf_sb[:, c * nblk * 10:(c + 1) * nblk * 10],
                            in_=hf[l - 1][c])
                    for (t0, g, qrt, q, co) in cfg.gplan:
                        icols = g * 128 // 16
                        _dma_gather_sbuf(
                            nc.gpsimd,
                            out_ap=xga[:, t0 * 10:(t0 + g) * 10],
                            in_ap=hf_sb[:, qrt * 4 * nblk * 10:
                                        (qrt + 1) * 4 * nblk * 10],
                            idxs_ap=gidx_sb[0:16, co:co + icols],
                            num_idxs=g * 128,
                            elem_size=10,
                            queue_num=q,
                            tokens_per_rank=128,
                            free_dim_per_rank=20)
                    if debug:
                        nc.sync.dma_start(out=dbg_xga[l - 1].ap(),
                                          in_=xga[:])
                    xg = xga[:].rearrange("p (t i) -> p t i", i=10)

                asum = node_pool.tile([128, nblk * oc], f32, tag="asum")
                prev = None  # (pbuf, r) pending scatter
                B = PSUM_BANK_F32 // d  # blocks per scatter-PSUM bank
                sc = {"agp": None, "blocks": []}

                def emit_scatter(pb_r, rp):
                    if sc["agp"] is None:
                        sc["agp"] = ag_pool.tile([128, B * d], f32,
                                                 tag="ag", name="agp")
                        sc["blocks"] = []
                    b = len(sc["blocks"])
                    agp = sc["agp"]
                    tls = tiles_of_block(rp)
                    for j, tl in enumerate(tls):
                        nc.tensor.matmul(
                            out=agp[:, b * d:(b + 1) * d],
                            lhsT=st_sb[:, tl * 128:(tl + 1) * 128],
                            rhs=pb_r[:, j * d:(j + 1) * d],
                            start=(j == 0), stop=(j == len(tls) - 1))
                    sc["blocks"].append(rp)
                    if len(sc["blocks"]) == B or rp == nblk - 1:
                        nb = len(sc["blocks"])
                        r0 = sc["blocks"][0]
                        nc.vector.reduce_sum(
                            out=asum[:, r0 * oc:(r0 + nb) * oc],
                            in_=agp[:, 0:nb * d].rearrange(
                                "p (q i) -> p q i", i=ic),
                            axis=AX.X)
                        sc["agp"] = None

                for r in range(nblk):
                    lane = (0, 2)[r % 2]
                    tls = tiles_of_block(r)
                    xtls = tls if l == 0 else xga_tiles_of_block(r)
                    pbuf = pb_pool.tile([128, tpb * d], bf16, tag="pb")
                    if lane != 0:
                        rbuf = pb_pool.tile([128, tpb * d], bf16, tag="rb")
                    t0 = 0
                    while t0 < tpb:
                        g = min(gmax, tpb - t0)
                        zp = zp_pool.tile([128, PSUM_BANK_F32], f32,
                                          tag="zp")
                        for j in range(g):
                            tl = tls[t0 + j]
                            nc.tensor.matmul(
                                out=zp[:, j * d:(j + 1) * d],
                                lhsT=so_sb[:, tl * 128:(tl + 1) * 128],
                                rhs=c123_sb[l][:],
                                start=True, stop=True)
                        if lane == 0:
                            for j in range(g):
                                t = t0 + j
                                nc.vector.scalar_tensor_tensor(
                                    out=pbuf[:, t * d:(t + 1) * d]
                                    .rearrange("p (o i) -> p o i",
                                               o=oc, i=ic),
                                    in0=zp[:, j * d:(j + 1) * d].rearrange(
                                        "p (o i) -> p o i", o=oc, i=ic),
                                    scalar=0.0,
                                    in1=xg[:, xtls[t]:xtls[t] + 1,
                                           :].to_broadcast([128, oc, ic]),
                                    op0=OP.max, op1=OP.mult)
                        else:
                            nc.scalar.activation(
                                out=rbuf[:, t0 * d:(t0 + g) * d],
                                in_=zp[:, 0:g * d],
                                func=mybir.ActivationFunctionType.Relu)
                            eng = nc.gpsimd if lane == 1 else nc.vector
                            for j in range(g):
                                t = t0 + j
                                eng.tensor_tensor(
                                    out=pbuf[:, t * d:(t + 1) * d]
                                    .rearrange("p (o i) -> p o i",
                                               o=oc, i=ic),
                                    in0=rbuf[:, t * d:(t + 1) * d]
                                    .rearrange("p (o i) -> p o i",
                                               o=oc, i=ic),
                                    in1=xg[:, xtls[t]:xtls[t] + 1,
                                           :].to_broadcast([128, oc, ic]),
                                    op=OP.mult)
                        t0 += g

                    # emit the PREVIOUS block's scatter so PE never
                    # head-blocks on this block's relu*x
                    if prev is not None:
                        emit_scatter(*prev)
                    prev = (pbuf, r)

                emit_scatter(*prev)
                prev = None

                # ---- node phase (batched over blocks) ------------------
                # 16-col stride per block so no matmul output crosses a
                # PSUM bank boundary (64*16 = exactly 2 banks)
                rt = rt_pool.tile([128, nblk * 16], f32, tag="rt")
                kr = 17 if l == 0 else 11
                for r in range(nblk):
                    nc.tensor.matmul(
                        out=rt[:, r * 16:r * 16 + oc],
                        lhsT=xt_sb[0:kr, r * 128:(r + 1) * 128],
                        rhs=rb_sb[l][:],
                        start=True, stop=True)
                hsum = node_pool.tile([128, nblk * oc], f32, tag="hsum")
                nc.vector.scalar_tensor_tensor(
                    out=hsum[:].rearrange("p (q o) -> p q o", o=oc),
                    in0=rt[:].rearrange("p (q s) -> p q s", s=16)
                    [:, :, 0:oc],
                    scalar=0.0,
                    in1=asum[:].rearrange("p (q o) -> p q o", o=oc),
                    op0=OP.add, op1=OP.add)
                hrel = node_pool.tile([128, nblk * oc], bf16, tag="hrel")
                nc.vector.tensor_scalar_max(out=hrel[:], in0=hsum[:],
                                            scalar1=0.0)
                if debug:
                    nc.sync.dma_start(out=dbg_h[l].ap(), in_=hrel[:])

                if l < 2:
                    nc.sync.dma_start(out=hc[l][:], in_=hrel[:])
                    nc.gpsimd.collective_compute(
                        "AllGather", OP.bypass, replica_groups=groups,
                        ins=[hc[l][:]], outs=[hf[l][:]])
                    for r in range(nblk):
                        tp = tp_pool.tile([16, 128], bf16, tag="tp")
                        nc.tensor.transpose(
                            out=tp[0:oc, :],
                            in_=hrel[:, r * oc:(r + 1) * oc],
                            identity=ident[:])
                        nc.scalar.copy(
                            out=xt_sb[0:oc, r * 128:(r + 1) * 128],
                            in_=tp[0:oc, :])
                else:
                    plp = tp_pool.tile([8, 16], f32, tag="tp", name="plp")
                    for r in range(nblk):
                        nc.tensor.matmul(
                            out=plp[:],
                            lhsT=ppool_sb[:, r * 8:(r + 1) * 8],
                            rhs=hrel[:, r * oc:(r + 1) * oc],
                            start=(r == 0), stop=(r == nblk - 1))
                    pool_sb = small_pool.tile([8, 16], f32, tag="pool_sb")
                    nc.scalar.copy(out=pool_sb[:], in_=plp[:])
                    nc.sync.dma_start(out=pool_in[:], in_=pool_sb[:])
                    nc.gpsimd.collective_compute(
                        "AllReduce", OP.add, replica_groups=groups,
                        ins=[pool_in[:]], outs=[pool_out[:]])
                    pool2 = small_pool.tile([8, 16], f32, tag="pool2")
                    nc.sync.dma_start(out=pool2[:], in_=pool_out[:])
                    outp = small_pool.tile([8, 16], f32, tag="outp")
                    nc.vector.tensor_scalar_mul(out=outp[:], in0=pool2[:],
                                                scalar1=cntr_sb[:, 0:1])
                    nc.sync.dma_start(out=out_d.ap(), in_=outp[:])

    nc.compile()
    return nc


# --------------------------------------------------------------------------
# host-side preparation
# --------------------------------------------------------------------------

def _assign_nodes(deg, n, npc):
    """LPT: nodes -> cores balancing in-edge load, <= npc nodes/core."""
    import heapq

    order = np.argsort(-deg, kind="stable")
    heap = [(0, c) for c in range(N_CORES)]
    heapq.heapify(heap)
    counts = np.zeros(N_CORES, np.int64)
    core_of = np.zeros(n, np.int64)
    spill = []
    for node in order:
        load, c = heapq.heappop(heap)
        core_of[node] = c
        counts[c] += 1
        load += int(deg[node])
        if counts[c] < npc:
            heapq.heappush(heap, (load, c))
        else:
            spill.append((load, c))
    return core_of


def _pack_blocks(nodes, wa, wb, nblk, cap, max_nodes):
    """Greedy 2D packing: nodes (sorted by wa+wb desc) into nblk blocks
    with per-block quartet loads <= cap and <= max_nodes nodes.
    Returns block assignment or None on failure."""
    la = np.zeros(nblk, np.int64)
    lb = np.zeros(nblk, np.int64)
    cnt = np.zeros(nblk, np.int64)
    blk_of = np.zeros(len(nodes), np.int64)
    order = np.argsort(-(wa + wb), kind="stable")
    for i in order:
        a, b = wa[i], wb[i]
        feas = (la + a <= cap) & (lb + b <= cap) & (cnt < max_nodes)
        if not feas.any():
            return None
        score = np.maximum(la + a, lb + b)
        score[~feas] = 1 << 60
        j = int(np.argmin(score))
        blk_of[i] = j
        la[j] += a
        lb[j] += b
        cnt[j] += 1
    return blk_of


def host_prep(cfg: Cfg, inputs: dict, aux=None):
    np_f32 = np.float32
    bf = ml_dtypes.bfloat16
    x = np.asarray(inputs["x"], np_f32)
    ef = np.asarray(inputs["edge_feat"], np_f32)
    et = np.asarray(inputs["edge_type"]).astype(np.int64)
    src = np.asarray(inputs["edge_src"]).astype(np.int64)
    dst = np.asarray(inputs["edge_dst"]).astype(np.int64)
    cell = np.asarray(inputs["cell_type"]).astype(np.int64)
    bids = np.asarray(inputs["batch_ids"]).astype(np.int64)

    n = cfg.n_nodes
    npad, nblk, tpb = cfg.npad, cfg.nblk, cfg.tpb
    ntiles, half, epad = cfg.ntiles, cfg.half, cfg.epad

    deg = np.bincount(dst, minlength=n).astype(np.int64)
    ncore, npart, nr = (inputs["_node_core"], inputs["_node_p"],
                        inputs["_node_r"])

    # folded weights per layer (o-major columns) + root/bias stack
    c123, rbs = [], []
    for l, (ic, oc) in enumerate(LAYER_DIMS):
        i = l + 1
        emb = np.asarray(inputs[f"emb{i}"], np_f32)
        wh = np.asarray(inputs[f"wh{i}"], np_f32)
        bh = np.asarray(inputs[f"bh{i}"], np_f32)
        wg = np.asarray(inputs[f"wg{i}"], np_f32)
        bg = np.asarray(inputs[f"bg{i}"], np_f32)
        c1 = emb * wh[0][None, :] + wg[0][None, :]
        c2 = emb * wh[1][None, :] + wg[1][None, :]
        c3 = emb * bh[None, :] + bg[None, :]
        m = np.concatenate([c1, c2, c3], axis=0)
        j = np.arange(ic * oc)
        o_, i_ = j // ic, j % ic
        c123.append(np.ascontiguousarray(m[:, i_ * oc + o_]).astype(bf))
        root = np.asarray(inputs[f"root{i}"], np_f32)
        bias = np.asarray(inputs[f"bias{i}"], np_f32)
        if l == 0:
            # xt row layout: feat 0-9, ones (row 10), feat 10-15 — so the
            # ones row is already in place for the layer 2-3 root matmuls
            rbs.append(np.vstack([root[0:10], bias[None, :],
                                  root[10:16]]).astype(bf))
        else:
            rbs.append(np.vstack([root, bias[None, :]]).astype(bf))

    e_core = ncore[dst]
    e_r = nr[dst]
    e_p = npart[dst]
    e_qrt = (ncore[src] // 4).astype(np.int64)

    # gather idx of a source node: local rank*(128) + partition
    g_idx = (((ncore[src] % 4) * nblk + nr[src]) * 128
             + npart[src]).astype(np.int64)

    msum = np.zeros((N_CORES * 8, 8), np.float32)
    for c in range(N_CORES):
        msum[c * 8 + np.arange(8), np.arange(8)] = 1.0
    msum = msum.astype(bf)

    gate = (cell == 1)
    cnt = np.bincount(bids[gate], minlength=8).astype(np_f32)
    cntr = (1.0 / np.maximum(cnt, 1.0)).reshape(8, 1).astype(np_f32)

    icols = G_TILES * 128 // 16

    in_maps = []
    for c in range(N_CORES):
        esel = np.where(e_core == c)[0]
        blk = e_r[esel]
        qrt = e_qrt[esel]
        # slot = tile*128 + pos within tile; tiles: Q0 block r -> 2r,2r+1
        # (positions 0..255), Q1 -> half+2r, half+2r+1
        order = np.lexsort((qrt, blk))
        esel = esel[order]
        blk = blk[order]
        qrt = qrt[order]
        # position within (block, quartet)
        key = blk * 2 + qrt
        bc = np.bincount(key, minlength=nblk * 2)
        assert bc.max() <= TQ * 128, (bc.max(), TQ * 128)
        within = np.arange(len(esel)) - np.repeat(
            np.concatenate([[0], np.cumsum(bc)[:-1]]), bc)
        tile0 = 4 * blk + 2 * qrt
        slots = (tile0 + within // 128) * 128 + within % 128

        et_c = et[esel]
        ef_c = ef[esel]

        so = np.zeros((K_SO, epad), np_f32)
        so[et_c, slots] = ef_c[:, 0]
        so[N_ET + et_c, slots] = ef_c[:, 1]
        so[2 * N_ET + et_c, slots] = 1.0

        st = np.zeros((epad, 128), np_f32)
        st[slots, e_p[esel]] = 1.0 / np.maximum(deg[dst[esel]], 1.0)
        st = st.reshape(ntiles, 128, 128).transpose(1, 0, 2)
        st = np.ascontiguousarray(st).reshape(128, ntiles * 128)

        xs = np.zeros((epad, 16), np_f32)
        xs[slots, :] = x[src[esel]]
        xs = xs.reshape(ntiles, 128, 16).transpose(1, 0, 2)
        xs = np.ascontiguousarray(xs).reshape(128, ntiles * 16)

        # gather idxs per slot (pad slots -> 0); permute from the
        # block-contiguous so/st order into xga quartet-half order
        gs = np.zeros(epad, np.int64)
        gs[slots] = g_idx[esel]
        S = np.empty(ntiles, np.int64)   # xga tile -> so tile
        rr = np.arange(nblk)
        S[2 * rr] = 4 * rr
        S[2 * rr + 1] = 4 * rr + 1
        S[half + 2 * rr] = 4 * rr + 2
        S[half + 2 * rr + 1] = 4 * rr + 3
        gs = gs.reshape(ntiles, 128)[S].ravel()
        gidx = np.zeros((128, cfg.idx_cols), np.int16)
        for (t0, g, qq, qnum, co) in cfg.gplan:
            ic_n = g * 128 // 16
            vals = gs[t0 * 128:(t0 + g) * 128]  # position i order
            w = np.zeros((16, ic_n), np.int16)
            ii = np.arange(g * 128)
            w[ii % 16, ii // 16] = vals.astype(np.int16)
            band = qnum * 32
            colr = slice(co, co + ic_n)
            gidx[band:band + 16, colr] = w
            gidx[band + 16:band + 32, colr] = w

        own = np.where(ncore == c)[0]
        ell = npart[own] * nblk + nr[own]

        xt1 = np.zeros((17, npad), np_f32)  # col r*128+p = node (p,r)
        cols = nr[own] * 128 + npart[own]
        xt1[0:10, cols] = x[own].T[0:10]
        xt1[10, :] = 1.0
        xt1[11:17, cols] = x[own].T[10:16]

        pp = np.zeros((npad, 8), np_f32)
        g = gate[own]
        pp[ell[g], bids[own][g]] = 1.0
        pp = pp.reshape(128, nblk * 8)

        if aux is not None:
            aux.append({"slots": slots, "esel": esel})
        in_maps.append({
            "so": so.astype(bf),
            "st": st.astype(bf),
            "xs1": xs.astype(bf),
            "gidx": gidx,
            "c123_0": c123[0], "c123_1": c123[1], "c123_2": c123[2],
            "rb_0": rbs[0], "rb_1": rbs[1], "rb_2": rbs[2],
            "xt1": xt1.astype(bf),
            "msum": msum,
            "ppool": pp.astype(bf),
            "cntr": cntr,
        })
    return in_maps


def prepare(inputs: dict, n_nodes, n_edges, npc):
    """Node->core->block assignment honoring per-quartet tile caps."""
    src = np.asarray(inputs["edge_src"]).astype(np.int64)
    dst = np.asarray(inputs["edge_dst"]).astype(np.int64)
    deg = np.bincount(dst, minlength=n_nodes).astype(np.int64)
    core = _assign_nodes(deg, n_nodes, npc)

    # per-node in-edge counts split by src quartet
    e_qrt = core[src] // 4
    wa_all = np.bincount(dst[e_qrt == 0], minlength=n_nodes)
    wb_all = np.bincount(dst[e_qrt == 1], minlength=n_nodes)

    p = np.zeros(n_nodes, np.int64)
    r = np.zeros(n_nodes, np.int64)
    for c in range(N_CORES):
        own = np.where(core == c)[0]
        blk = _pack_blocks(own, wa_all[own], wb_all[own], N_BLK,
                           TQ * 128, 128)
        assert blk is not None, f"2D block packing failed on core {c}"
        r[own] = blk
        # partition slot within block
        for b in range(N_BLK):
            sel = own[blk == b]
            p[sel] = np.arange(len(sel))

    cfg = Cfg(n_nodes, n_edges, npc)
    inputs = dict(inputs)
    inputs["_node_core"] = core
    inputs["_node_p"] = p
    inputs["_node_r"] = r
    return cfg, inputs


def kernel(**inputs) -> np.ndarray:
    from concourse.bass_utils import run_bass_kernel_spmd

    cfg, inputs2 = prepare(inputs, 50000, 250000, 6250)
    key = ("v4", cfg.ntiles)
    if key not in _BUILD_CACHE:
        _BUILD_CACHE[key] = build_nc(cfg)
    nc = _BUILD_CACHE[key]
    in_maps = host_prep(cfg, inputs2)
    res = run_bass_kernel_spmd(nc, in_maps, list(range(N_CORES)))
    return np.asarray(res.results[0]["out"], np.float32)
